# revision 1
# baseline (speedup 1.0000x reference)
"""Causal self-attention (B=2, T=2048, C=1024, H=16) on 8 Trainium2 NeuronCores.

Sharding (Megatron-style, chosen per hint): core c handles batch b = c//4 and
head group g = c%4 (4 heads each).  c_attn is column-parallel (each core gets
the 3x256 q/k/v columns for its heads), c_proj is row-parallel (each core gets
the 256 rows for its heads) and the 4 partial outputs per batch are summed on
the host (the row-parallel all-reduce), plus b_proj.

Per-core dataflow (bf16 matmul operands, fp32 PSUM accumulation):
  x^T [C, T] is pre-transposed on host, so contraction dims always sit on
  SBUF partitions:
   1. QT/KT [d, t] (d-major, head pairs packed on 128 partitions) and
      V [t, d] (t-major, with a ones column appended -> fused softmax denom).
   2. ST tile [j, i] = (K Q^T) per head, causal-windowed; triangular additive
      mask on diagonal 128-blocks; exp on ScalarE with fused 1/sqrt(64) scale;
      software-pipelined two ST-pairs ahead of the YT accumulation so the PE
      never idles on ScalarE (keeps the HAM clock-gate warm).
   3. YT [e, i] += V_aug^T @ P accumulated over j-blocks in PSUM; row 64 is
      the softmax denominator (ones column of V_aug).
   4. Unnormalized Y + denominator rows staged to SBUF immediately (frees
      PSUM in ~1us); denominators are stacked via DMA, batch-reciprocaled
      once per head pair, DMA-broadcast via a DRAM round-trip, and applied
      in place; then out[i, :] = sum_ho YT^T @ W_proj.
"""

import os
import sys
import types
from contextlib import ExitStack

import ml_dtypes
import numpy as np

for _p in ("/opt/trn_rl_repo",):
    if os.path.isdir(_p) and _p not in sys.path:
        sys.path.append(_p)
os.environ.setdefault("JAX_PLATFORMS", "cpu")

import concourse.bass as bass
import concourse.tile as tile
from concourse import bacc, mybir
from concourse.bass_utils import run_bass_kernel_spmd

B, T, C, H = 2, 2048, 1024, 16
P = 128
CO = C // P          # 8 contraction blocks for the qkv projection
HL = H // 4          # 4 local heads per core
D = C // H           # 64
NEG = -1.0e30
F32 = mybir.dt.float32
F32R = mybir.dt.float32r
BF16 = mybir.dt.bfloat16
EXPF = mybir.ActivationFunctionType.Exp
ADD = mybir.AluOpType.add
MULT = mybir.AluOpType.mult

_CACHE = {}


def _install_ntff_hook():
    """Agent image's antenv lacks axon_hooks; recreate so trace=True works."""
    try:
        from antenv import axon_hooks  # noqa: F401
        return
    except ImportError:
        pass
    try:
        import antenv
        from trn_agent_boot.trn_boot import _ntff_profile_via_ctypes
    except ImportError:
        return
    mod = types.ModuleType("antenv.axon_hooks")
    _hook = [None]
    mod.set_axon_ntff_profile_hook = lambda h: _hook.__setitem__(0, h)
    mod.get_axon_ntff_profile_hook = lambda: _hook[0]
    sys.modules["antenv.axon_hooks"] = mod
    antenv.axon_hooks = mod
    so = "/opt/axon/libaxon_pjrt.so"
    if os.path.exists(so):
        mod.set_axon_ntff_profile_hook(_ntff_profile_via_ctypes(so))


def build_module():
    nc = bacc.Bacc("TRN2", target_bir_lowering=False, debug=False, num_devices=8)

    xt_d = nc.dram_tensor("xt", [C, T], BF16, kind="ExternalInput").ap()
    wq_d = nc.dram_tensor("wq", [C, 256], BF16, kind="ExternalInput").ap()
    wk_d = nc.dram_tensor("wk", [C, 256], BF16, kind="ExternalInput").ap()
    wv_d = nc.dram_tensor("wv", [C, 256], BF16, kind="ExternalInput").ap()
    wp_d = nc.dram_tensor("wp", [256, C], BF16, kind="ExternalInput").ap()
    bq_d = nc.dram_tensor("bq", [256], F32, kind="ExternalInput").ap()
    bk_d = nc.dram_tensor("bk", [256], F32, kind="ExternalInput").ap()
    bv_d = nc.dram_tensor("bv", [256], F32, kind="ExternalInput").ap()
    tri_d = nc.dram_tensor("tri", [P, P], F32, kind="ExternalInput").ap()
    ones_d = nc.dram_tensor("onesd", [T // P * HL], BF16, kind="ExternalInput").ap()
    out_d = nc.dram_tensor("out", [T, C], F32, kind="ExternalOutput").ap()

    with tile.TileContext(nc) as tc, ExitStack() as ctx:
        const = ctx.enter_context(tc.tile_pool(name="const", bufs=1))
        s1w = ctx.enter_context(tc.tile_pool(name="s1w", bufs=1))
        # PSUM: 8 banks of [128, 512]f32 total.  acc(2) + stp(3) + ytp(3).
        psA = ctx.enter_context(tc.tile_pool(name="psA", bufs=2, space="PSUM"))
        psS = ctx.enter_context(tc.tile_pool(name="psS", bufs=4, space="PSUM"))
        psY = ctx.enter_context(tc.tile_pool(name="psY", bufs=2, space="PSUM"))
        ppool = ctx.enter_context(tc.tile_pool(name="ppool", bufs=6))
        rpool = ctx.enter_context(tc.tile_pool(name="rpool", bufs=4))
        opool = ctx.enter_context(tc.tile_pool(name="opool", bufs=3))
        dpool = ctx.enter_context(tc.tile_pool(name="dpool", bufs=4, space="DRAM"))

        # ---- persistent SBUF tensors -------------------------------------
        qt = const.tile([P, 2, T], BF16, tag="qt")     # [d, do, t]; head pair per do
        kt = const.tile([P, 2, T], BF16, tag="kt")
        vsb = const.tile([P, T // P, HL, 66], BF16, tag="vsb")  # [tp, to, l, 1|V|1]
        yt2 = const.tile([P, 2, T], BF16, tag="yt2")   # Y^T (unnorm, then scaled)
        wp_sb = const.tile([P, 2, C], BF16, tag="wp")
        tri_sb = const.tile([P, P], F32, tag="tri")
        bq_sb = const.tile([P, 2], F32, tag="bq")
        bk_sb = const.tile([P, 2], F32, tag="bk")
        bv_sb = const.tile([P, 256], F32, tag="bv")
        # stacked softmax denominators, 8 rows per ho: row = ib*2 + hp
        den = [const.tile([8, 512], F32, tag="den", name=f"den{i}") for i in range(2)]
        rden = [const.tile([8, 512], F32, tag="rden", name=f"rden{i}")
                for i in range(2)]

        xt_sb = s1w.tile([P, CO, T], BF16, tag="xt")
        wq_sb = s1w.tile([P, CO, 256], BF16, tag="wq")
        wk_sb = s1w.tile([P, CO, 256], BF16, tag="wk")
        wv_sb = s1w.tile([P, CO, 256], BF16, tag="wv")

        # ---- input DMA (split per contraction block so PE can start early)
        xt_r = xt_d.rearrange("(co p) t -> p co t", p=P)
        wq_r = wq_d.rearrange("(co p) d -> p co d", p=P)
        wk_r = wk_d.rearrange("(co p) d -> p co d", p=P)
        wv_r = wv_d.rearrange("(co p) d -> p co d", p=P)
        for co in range(CO):
            nc.sync.dma_start(wq_sb[:, co], wq_r[:, co])
            nc.sync.dma_start(wk_sb[:, co], wk_r[:, co])
            nc.sync.dma_start(wv_sb[:, co], wv_r[:, co])
            nc.sync.dma_start(xt_sb[:, co], xt_r[:, co])
        nc.sync.dma_start(wp_sb[:], wp_d.rearrange("(ho p) n -> p ho n", p=P))
        nc.sync.dma_start(tri_sb[:], tri_d)
        nc.sync.dma_start(bq_sb[:], bq_d.rearrange("(do p) -> p do", p=P))
        nc.sync.dma_start(bk_sb[:], bk_d.rearrange("(do p) -> p do", p=P))
        nc.sync.dma_start(
            bv_sb[:],
            bass.AP(tensor=bv_d.tensor, offset=bv_d.offset,
                    ap=[[0, P]] + list(bv_d.ap)),
        )
        nc.vector.memset(vsb[:, :, :, 65:66], 1.0)

        # ---- stage 1: qkv projection -------------------------------------
        def emit_qk_group(w_sb, b_sb, dst, do, t4):
            # QT/KT d-major: psum[d, t] = W[:, dcols]^T @ x^T
            ps = psA.tile([P, 512], F32, tag="acc", name="qkps")
            for co in range(CO):
                nc.tensor.matmul(
                    ps[:],
                    lhsT=w_sb[:, co, do * P:(do + 1) * P],
                    rhs=xt_sb[:, co, t4 * 512:(t4 + 1) * 512],
                    start=(co == 0), stop=(co == CO - 1),
                )
            nc.vector.tensor_scalar_add(
                dst[:, do, t4 * 512:(t4 + 1) * 512], ps[:], b_sb[:, do:do + 1])

        def emit_v_group(to):
            # V t-major: psum[t, d] = x^T-block^T @ Wv
            ps = psA.tile([P, 512], F32, tag="acc", name="vps")[:, 0:256]
            for co in range(CO):
                nc.tensor.matmul(
                    ps[:],
                    lhsT=xt_sb[:, co, to * P:(to + 1) * P],
                    rhs=wv_sb[:, co, :],
                    start=(co == 0), stop=(co == CO - 1),
                )
            nc.vector.tensor_tensor(
                vsb[:, to, :, 1:65],
                ps[:].rearrange("p (l e) -> p l e", l=HL),
                bv_sb[:].rearrange("p (l e) -> p l e", l=HL),
                op=ADD,
            )

        for w_sb, b_sb, dst in ((wq_sb, bq_sb, qt), (wk_sb, bk_sb, kt)):
            for do in range(2):
                for t4 in range(T // 512):
                    emit_qk_group(w_sb, b_sb, dst, do, t4)
        for to in range(T // P):
            emit_v_group(to)

        # ---- stages 2-4: attention, head pair (2*ho, 2*ho+1) -------------
        NB = T // 512                       # 4 i-blocks of 512
        for ho in range(2):
            for ib in range(NB):
                ytp = [psY.tile([P, 512], F32, tag="ytp", name=f"ytp_{hp}")
                       for hp in range(2)]
                njb = 4 * ib + 4

                def win(jb):
                    r = jb - 4 * ib
                    i0 = jb * P if r >= 0 else ib * 512
                    return r, i0, (ib + 1) * 512 - i0

                pts = {}

                def emit_st(jb):
                    r, i0, N = win(jb)
                    jsl = slice(jb * P, (jb + 1) * P)
                    pair = []
                    for hp in range(2):
                        pb = hp * 64
                        stp = psS.tile([P, 512], F32, tag="stp")
                        nc.tensor.matmul(
                            stp[:, :N], lhsT=kt[pb:pb + 64, ho, jsl],
                            rhs=qt[pb:pb + 64, ho, i0:i0 + N],
                            start=True, stop=True)
                        if r >= 0:
                            nc.vector.tensor_tensor(
                                stp[:, 0:P], stp[:, 0:P], tri_sb[:], op=ADD)
                        pt = ppool.tile([P, 512], BF16, tag="pt")
                        nc.scalar.activation(pt[:, :N], stp[:, :N], EXPF,
                                             scale=float(1.0 / np.sqrt(D)))
                        pair.append(pt)
                    pts[jb] = pair

                def emit_yt(jb):
                    _, i0, N = win(jb)
                    f0 = i0 - ib * 512
                    last = jb == njb - 1
                    pair = pts.pop(jb)
                    for hp in range(2):
                        nc.tensor.matmul(
                            ytp[hp][0:65, f0:f0 + N],
                            lhsT=vsb[:, jb, 2 * ho + hp, 1:66],
                            rhs=pair[hp][:, :N], start=(jb == 0), stop=last)

                # software pipeline: keep PE two ST-pairs ahead of the
                # exp-dependent YT accumulations so it never idles on ScalarE
                emit_st(0)
                if njb > 1:
                    emit_st(1)
                for jb in range(njb):
                    if jb + 2 < njb:
                        emit_st(jb + 2)
                    emit_yt(jb)
                # epilogue: stage unnormalized Y + denominator row, free PSUM
                iw = slice(ib * 512, (ib + 1) * 512)
                for hp in range(2):
                    nc.vector.tensor_copy(
                        yt2[hp * 64:hp * 64 + 64, ho, iw], ytp[hp][0:64, :])
                    dr = rpool.tile([1, 512], F32, tag="dr", name=f"dr{hp}")
                    nc.vector.tensor_copy(dr[:], ytp[hp][64:65, :])
                    ri = ib * 2 + hp
                    nc.sync.dma_start(den[ho][ri:ri + 1, :], dr[:])
            # batched reciprocal for this ho, then deferred normalize
            nc.vector.reciprocal(rden[ho][:], den[ho][:])
            dscr = dpool.tile([8, 512], F32, tag="dscr", name=f"dscr{ho}")
            nc.sync.dma_start(dscr[:], rden[ho][:])
            for ib in range(NB):
                iw = slice(ib * 512, (ib + 1) * 512)
                for hp in range(2):
                    src = dscr[ib * 2 + hp, :]
                    pb = hp * 64
                    rdb = rpool.tile([P, 512], F32, tag="rdb")
                    nc.sync.dma_start(
                        rdb[pb:pb + 64, :],
                        bass.AP(tensor=src.tensor, offset=src.offset,
                                ap=[[0, 64]] + list(src.ap)))
                    ysl = yt2[pb:pb + 64, ho, iw]
                    nc.vector.tensor_tensor(ysl, ysl, rdb[pb:pb + 64, :], op=MULT)

        # ---- stage 5: output projection (row-parallel partial) -----------
        for i1 in range(T // P):
            isl = slice(i1 * P, (i1 + 1) * P)
            for nh in range(C // 512):
                nsl = slice(nh * 512, (nh + 1) * 512)
                ps = psA.tile([P, 512], F32, tag="acc")
                for ho in range(2):
                    nc.tensor.matmul(
                        ps[:], lhsT=yt2[:, ho, isl], rhs=wp_sb[:, ho, nsl],
                        start=(ho == 0), stop=(ho == 1))
                ot = opool.tile([P, 512], F32, tag="ot")
                nc.vector.tensor_copy(ot[:], ps[:])
                nc.sync.dma_start(out_d[isl, nsl], ot[:])

    nc.compile()
    return nc


def _get_module():
    if "nc" not in _CACHE:
        _CACHE["nc"] = build_module()
    return _CACHE["nc"]


def _make_in_maps(x, W_attn, b_attn, W_proj):
    tri = np.where(np.arange(P)[None, :] >= np.arange(P)[:, None],
                   np.float32(0.0), np.float32(NEG)).astype(np.float32)
    bf = ml_dtypes.bfloat16
    in_maps = []
    for core in range(8):
        b, g = divmod(core, 4)
        cs = slice(g * 256, (g + 1) * 256)
        in_maps.append({
            "xt": np.ascontiguousarray(x[b].T).astype(bf),
            "wq": np.ascontiguousarray(W_attn[:, g * 256:(g + 1) * 256]).astype(bf),
            "wk": np.ascontiguousarray(
                W_attn[:, C + g * 256:C + (g + 1) * 256]).astype(bf),
            "wv": np.ascontiguousarray(
                W_attn[:, 2 * C + g * 256:2 * C + (g + 1) * 256]).astype(bf),
            "wp": np.ascontiguousarray(W_proj[cs, :]).astype(bf),
            "bq": np.ascontiguousarray(b_attn[cs]),
            "bk": np.ascontiguousarray(b_attn[C + g * 256:C + (g + 1) * 256]),
            "bv": np.ascontiguousarray(b_attn[2 * C + g * 256:2 * C + (g + 1) * 256]),
            "tri": tri,
            "onesd": np.ones(T // P * HL, bf),
        })
    return in_maps


def _gather(results, b_proj):
    y = np.empty((B, T, C), np.float32)
    for b in range(B):
        acc = results[4 * b]["out"].astype(np.float32).copy()
        for g in range(1, 4):
            acc += results[4 * b + g]["out"]
        y[b] = acc + b_proj[None, :].astype(np.float32)
    return y


def kernel(x, W_attn, b_attn, W_proj, b_proj, _trace=False):
    x = np.asarray(x, np.float32)
    W_attn = np.asarray(W_attn, np.float32)
    b_attn = np.asarray(b_attn, np.float32)
    W_proj = np.asarray(W_proj, np.float32)
    b_proj = np.asarray(b_proj, np.float32)

    nc = _get_module()
    in_maps = _make_in_maps(x, W_attn, b_attn, W_proj)
    kw = {}
    if _trace:
        _install_ntff_hook()
        kw = dict(trace=True)
    res = run_bass_kernel_spmd(nc, in_maps, core_ids=list(range(8)), **kw)
    out = _gather(res.results, b_proj)
    if _trace:
        return out, res
    return out



# revision 4
# speedup vs baseline: 1.1648x; 1.1648x over previous
"""Causal self-attention (B=2, T=2048, C=1024, H=16) on 8 Trainium2 NeuronCores.

Sharding (Megatron-style, per hint): core c handles batch b = c//4 and head
group g = c%4 (4 heads each).  c_attn is column-parallel (each core gets the
3x256 q/k/v columns for its heads), c_proj is row-parallel (each core gets
the 256 rows for its heads); the 4 partial outputs per batch are summed on
the host (the row-parallel all-reduce), plus b_proj.  Partials ship as bf16.

Single fused software-pipelined stream per core (keeps the PE HAM clock-gate
warm by never letting the Tensor engine idle during the ScalarE-bound
softmax phase):

  1. qkv projection for token block t4 is emitted as *filler* inside the
     attention j-loop of block t4-1, and the output projection of row block
     ib-1 likewise fills the attention of ib.  x^T is DMA'd in (co, t4)
     chunks so the first matmul starts ~2MB into the transfer.
  2. ST pair tiles [j, 2hp x i] live in one 2-bank PSUM tile so ONE ScalarE
     ACTIVATE (exp, fused 1/sqrt(D) scale) covers both heads of a pair --
     halves the fixed 352-cycle ACTIVATE overhead.  Causal masking is a
     multiplicative bf16 0/1 triangle on the post-exp tile (cheap DVE op).
  3. The two ST matmuls of a pair use K=64 row-halves of the PE array
     (distinct tile_position row groups) so they stream concurrently.
  4. YT[e, i] += V_aug^T @ P accumulates both heads into one 2-bank PSUM
     tile; row 64 is the softmax denominator (ones column of V_aug).
  5. Normalization: DVE reciprocal of the two denominator rows -> one bf16
     row, broadcast across 64 partitions by a K=1 PE matmul (two concurrent
     column-tiles), then a fused PSUMxPSUM multiply writes normalized Y^T
     straight to SBUF.  No DRAM round-trip.
"""

import os
import sys
import types
from collections import deque
from contextlib import ExitStack

import ml_dtypes
import numpy as np

for _p in ("/opt/trn_rl_repo",):
    if os.path.isdir(_p) and _p not in sys.path:
        sys.path.append(_p)
os.environ.setdefault("JAX_PLATFORMS", "cpu")

import concourse.bass as bass
import concourse.tile as tile
from concourse import bacc, mybir
from concourse.bass_utils import run_bass_kernel_spmd

B, T, C, H = 2, 2048, 1024, 16
P = 128
CO = C // P          # 8 contraction blocks for the qkv projection
HL = H // 4          # 4 local heads per core
D = C // H           # 64
F32 = mybir.dt.float32
BF16 = mybir.dt.bfloat16
EXPF = mybir.ActivationFunctionType.Exp
ADD = mybir.AluOpType.add
MULT = mybir.AluOpType.mult

_CACHE = {}


def _install_ntff_hook():
    """Agent image's antenv lacks axon_hooks; recreate so trace=True works."""
    try:
        from antenv import axon_hooks  # noqa: F401
        return
    except ImportError:
        pass
    try:
        import antenv
        from trn_agent_boot.trn_boot import _ntff_profile_via_ctypes
    except ImportError:
        return
    mod = types.ModuleType("antenv.axon_hooks")
    _hook = [None]
    mod.set_axon_ntff_profile_hook = lambda h: _hook.__setitem__(0, h)
    mod.get_axon_ntff_profile_hook = lambda: _hook[0]
    sys.modules["antenv.axon_hooks"] = mod
    antenv.axon_hooks = mod
    so = "/opt/axon/libaxon_pjrt.so"
    if os.path.exists(so):
        mod.set_axon_ntff_profile_hook(_ntff_profile_via_ctypes(so))


def build_module():
    nc = bacc.Bacc("TRN2", target_bir_lowering=False, debug=False, num_devices=8)

    xt_d = nc.dram_tensor("xt", [C, T], BF16, kind="ExternalInput").ap()
    wq_d = nc.dram_tensor("wq", [C, 256], BF16, kind="ExternalInput").ap()
    wk_d = nc.dram_tensor("wk", [C, 256], BF16, kind="ExternalInput").ap()
    wv_d = nc.dram_tensor("wv", [C, 256], BF16, kind="ExternalInput").ap()
    wp_d = nc.dram_tensor("wp", [256, C], BF16, kind="ExternalInput").ap()
    bq_d = nc.dram_tensor("bq", [256], F32, kind="ExternalInput").ap()
    bk_d = nc.dram_tensor("bk", [256], F32, kind="ExternalInput").ap()
    bv_d = nc.dram_tensor("bv", [256], F32, kind="ExternalInput").ap()
    tri_d = nc.dram_tensor("tri", [P, 2 * P], BF16, kind="ExternalInput").ap()
    out_d = nc.dram_tensor("out", [T, C], BF16, kind="ExternalOutput").ap()

    NB = T // 512                       # 4 i-blocks of 512

    with tile.TileContext(nc) as tc, ExitStack() as ctx:
        const = ctx.enter_context(tc.tile_pool(name="const", bufs=1))
        s1w = ctx.enter_context(tc.tile_pool(name="s1w", bufs=1))
        # PSUM: 8 banks of [128, 512]f32.  psA(2) + psS(2x2) + psY(2) = 8.
        psA = ctx.enter_context(tc.tile_pool(name="psA", bufs=2, space="PSUM"))
        psS = ctx.enter_context(tc.tile_pool(name="psS", bufs=2, space="PSUM"))
        psY = ctx.enter_context(tc.tile_pool(name="psY", bufs=1, space="PSUM"))
        ppool = ctx.enter_context(tc.tile_pool(name="ppool", bufs=3))
        drp = ctx.enter_context(tc.tile_pool(name="drp", bufs=2))
        opool = ctx.enter_context(tc.tile_pool(name="opool", bufs=3))

        # ---- persistent SBUF tensors -------------------------------------
        qt = const.tile([P, 2, T], BF16, tag="qt")     # [d, ho, t]; head pair per ho
        kt = const.tile([P, 2, T], BF16, tag="kt")
        vsb = const.tile([P, T // P, HL, 66], BF16, tag="vsb")  # [tp, to, l, 1|V|1]
        yt2 = const.tile([P, 2, T], BF16, tag="yt2")   # normalized Y^T
        wp_sb = const.tile([P, 2, C], BF16, tag="wp")
        tri_sb = const.tile([P, 2 * P], BF16, tag="tri")   # 0/1 causal pair mask
        ones64 = const.tile([1, D], BF16, tag="ones64")
        bq_sb = const.tile([P, 2], F32, tag="bq")
        bk_sb = const.tile([P, 2], F32, tag="bk")
        bv_sb = const.tile([P, 256], F32, tag="bv")

        xt_sb = s1w.tile([P, CO, T], BF16, tag="xt")
        wq_sb = s1w.tile([P, CO, 256], BF16, tag="wq")
        wk_sb = s1w.tile([P, CO, 256], BF16, tag="wk")
        wv_sb = s1w.tile([P, CO, 256], BF16, tag="wv")

        # ---- input DMA, ordered by first use -----------------------------
        xt_r = xt_d.rearrange("(co p) t -> p co t", p=P)
        wq_r = wq_d.rearrange("(co p) d -> p co d", p=P)
        wk_r = wk_d.rearrange("(co p) d -> p co d", p=P)
        wv_r = wv_d.rearrange("(co p) d -> p co d", p=P)
        nc.sync.dma_start(tri_sb[:], tri_d)
        nc.sync.dma_start(bq_sb[:], bq_d.rearrange("(do p) -> p do", p=P))
        nc.sync.dma_start(bk_sb[:], bk_d.rearrange("(do p) -> p do", p=P))
        for co in range(CO):
            nc.sync.dma_start(wq_sb[:, co], wq_r[:, co])
            nc.sync.dma_start(wk_sb[:, co], wk_r[:, co])
        for co in range(CO):
            nc.sync.dma_start(xt_sb[:, co, 0:512], xt_r[:, co, 0:512])
        for co in range(CO):
            nc.sync.dma_start(wv_sb[:, co], wv_r[:, co])
        nc.sync.dma_start(
            bv_sb[:],
            bass.AP(tensor=bv_d.tensor, offset=bv_d.offset,
                    ap=[[0, P]] + list(bv_d.ap)),
        )
        for t4 in range(1, NB):
            for co in range(CO):
                nc.sync.dma_start(xt_sb[:, co, t4 * 512:(t4 + 1) * 512],
                                  xt_r[:, co, t4 * 512:(t4 + 1) * 512])
        nc.sync.dma_start(wp_sb[:], wp_d.rearrange("(ho p) n -> p ho n", p=P))
        nc.vector.memset(vsb[:, :, :, 65:66], 1.0)
        nc.vector.memset(ones64[:], 1.0)

        # ---- stage 1: qkv projection -------------------------------------
        def qk_group(w_sb, b_sb, dst, do, t4):
            # QT/KT d-major: psum[d, t] = W[:, dcols]^T @ x^T
            ps = psA.tile([P, 512], F32, tag="acc", name="qkps")
            for co in range(CO):
                nc.tensor.matmul(
                    ps[:],
                    lhsT=w_sb[:, co, do * P:(do + 1) * P],
                    rhs=xt_sb[:, co, t4 * 512:(t4 + 1) * 512],
                    start=(co == 0), stop=(co == CO - 1),
                )
            nc.vector.tensor_scalar_add(
                dst[:, do, t4 * 512:(t4 + 1) * 512], ps[:], b_sb[:, do:do + 1])

        def v_group(to):
            # V t-major: psum[t, d] = x^T-block^T @ Wv
            ps = psA.tile([P, 512], F32, tag="acc", name="vps")[:, 0:256]
            for co in range(CO):
                nc.tensor.matmul(
                    ps[:],
                    lhsT=xt_sb[:, co, to * P:(to + 1) * P],
                    rhs=wv_sb[:, co, :],
                    start=(co == 0), stop=(co == CO - 1),
                )
            nc.vector.tensor_tensor(
                vsb[:, to, :, 1:65],
                ps[:].rearrange("p (l e) -> p l e", l=HL),
                bv_sb[:].rearrange("p (l e) -> p l e", l=HL),
                op=ADD,
            )

        def qkv_emitters(t4):
            ems = []
            for do in range(2):
                ems.append(lambda do=do, t4=t4: qk_group(wq_sb, bq_sb, qt, do, t4))
                ems.append(lambda do=do, t4=t4: qk_group(wk_sb, bk_sb, kt, do, t4))
            for to in range(4 * t4, 4 * t4 + 4):
                ems.append(lambda to=to: v_group(to))
            return ems

        # ---- stage 5: output projection (row-parallel partial) -----------
        def proj_group(i1, nh):
            isl = slice(i1 * P, (i1 + 1) * P)
            nsl = slice(nh * 512, (nh + 1) * 512)
            ps = psA.tile([P, 512], F32, tag="acc", name="prps")
            for ho in range(2):
                nc.tensor.matmul(
                    ps[:], lhsT=yt2[:, ho, isl], rhs=wp_sb[:, ho, nsl],
                    start=(ho == 0), stop=(ho == 1))
            ot = opool.tile([P, 512], BF16, tag="ot")
            nc.vector.tensor_copy(ot[:], ps[:])
            nc.sync.dma_start(out_d[isl, nsl], ot[:])

        def proj_emitters(ib):
            return [lambda i1=i1, nh=nh: proj_group(i1, nh)
                    for i1 in range(4 * ib, 4 * ib + 4) for nh in range(2)]

        # ---- stages 2-4: attention for head pair ho, row block ib --------
        tri3 = tri_sb.rearrange("p (h n) -> p h n", h=2)

        def attention(ho, ib, fillers, after_prologue=None):
            njb = 4 * ib + 4
            ytp = psY.tile([P, 1024], F32, tag="ytp")

            def win(jb):
                r = jb - 4 * ib
                i0 = jb * P if r >= 0 else ib * 512
                return r, i0, (ib + 1) * 512 - i0

            pts = {}

            def st_pair(jb):
                r, i0, N = win(jb)
                jsl = slice(jb * P, (jb + 1) * P)
                stp = psS.tile([P, 1024], F32, tag="stp")
                for hp in range(2):
                    pb = hp * 64
                    nc.tensor.matmul(
                        stp[:, hp * 512:hp * 512 + N],
                        lhsT=kt[pb:pb + 64, ho, jsl],
                        rhs=qt[pb:pb + 64, ho, i0:i0 + N],
                        start=True, stop=True)
                pt = ppool.tile([P, 2, 512], BF16, tag="pt")
                nc.scalar.activation(
                    pt[:, :, :N],
                    stp.rearrange("p (h n) -> p h n", h=2)[:, :, :N],
                    EXPF, scale=float(1.0 / np.sqrt(D)))
                if r >= 0:
                    nc.vector.tensor_tensor(
                        pt[:, :, 0:P], pt[:, :, 0:P], tri3[:], op=MULT)
                pts[jb] = pt

            def yt_pair(jb):
                _, i0, N = win(jb)
                f0 = i0 - ib * 512
                last = jb == njb - 1
                pt = pts.pop(jb)
                for hp in range(2):
                    nc.tensor.matmul(
                        ytp[0:65, hp * 512 + f0:hp * 512 + f0 + N],
                        lhsT=vsb[:, jb, 2 * ho + hp, 1:66],
                        rhs=pt[:, hp, :N], start=(jb == 0), stop=last)

            st_pair(0)
            if njb > 1:
                st_pair(1)
            if after_prologue is not None:
                after_prologue()
            for jb in range(njb):
                if jb + 2 < njb:
                    st_pair(jb + 2)
                if fillers:
                    fillers.popleft()()
                yt_pair(jb)

            def normalize():
                dr = drp.tile([1, 1024], BF16, tag="dr")
                with nc.allow_low_precision(reason="bf16 softmax denom"):
                    nc.vector.reciprocal(dr[:], ytp[64:65, :])
                rps = psA.tile([P, 512], F32, tag="acc", name="rps")
                for hp in range(2):
                    nc.tensor.matmul(
                        rps[hp * 64:hp * 64 + 64, :],
                        lhsT=ones64[0:1, :],
                        rhs=dr[0:1, hp * 512:(hp + 1) * 512],
                        start=True, stop=True)
                rsb = drp.tile([P, 512], BF16, tag="rsb")
                nc.vector.tensor_copy(rsb[:], rps[:])
                iw = slice(ib * 512, (ib + 1) * 512)
                for hp in range(2):
                    nc.vector.tensor_tensor(
                        yt2[hp * 64:hp * 64 + 64, ho, iw],
                        ytp[0:64, hp * 512:(hp + 1) * 512],
                        rsb[hp * 64:hp * 64 + 64, :], op=MULT)

            return normalize

        # ---- fused pipeline ----------------------------------------------
        for em in qkv_emitters(0):
            em()
        filler = deque()
        pending_norm = None
        for ib in range(NB):
            if ib + 1 < NB:
                filler.extend(qkv_emitters(ib + 1))
            if ib >= 1:
                filler.extend(proj_emitters(ib - 1))
            for ho in range(2):
                pending_norm = attention(ho, ib, filler,
                                         after_prologue=pending_norm)
            while filler:
                filler.popleft()()
        pending_norm()
        for em in proj_emitters(NB - 1):
            em()

    nc.compile()
    return nc


def _get_module():
    if "nc" not in _CACHE:
        _CACHE["nc"] = build_module()
    return _CACHE["nc"]


def _make_in_maps(x, W_attn, b_attn, W_proj):
    tri1 = np.where(np.arange(P)[None, :] >= np.arange(P)[:, None],
                    np.float32(1.0), np.float32(0.0))
    bf = ml_dtypes.bfloat16
    tri = np.concatenate([tri1, tri1], axis=1).astype(bf)
    in_maps = []
    for core in range(8):
        b, g = divmod(core, 4)
        cs = slice(g * 256, (g + 1) * 256)
        in_maps.append({
            "xt": np.ascontiguousarray(x[b].T).astype(bf),
            "wq": np.ascontiguousarray(W_attn[:, g * 256:(g + 1) * 256]).astype(bf),
            "wk": np.ascontiguousarray(
                W_attn[:, C + g * 256:C + (g + 1) * 256]).astype(bf),
            "wv": np.ascontiguousarray(
                W_attn[:, 2 * C + g * 256:2 * C + (g + 1) * 256]).astype(bf),
            "wp": np.ascontiguousarray(W_proj[cs, :]).astype(bf),
            "bq": np.ascontiguousarray(b_attn[cs]),
            "bk": np.ascontiguousarray(b_attn[C + g * 256:C + (g + 1) * 256]),
            "bv": np.ascontiguousarray(b_attn[2 * C + g * 256:2 * C + (g + 1) * 256]),
            "tri": tri,
        })
    return in_maps


def _gather(results, b_proj):
    y = np.empty((B, T, C), np.float32)
    for b in range(B):
        acc = results[4 * b]["out"].astype(np.float32)
        for g in range(1, 4):
            acc = acc + results[4 * b + g]["out"].astype(np.float32)
        y[b] = acc + b_proj[None, :].astype(np.float32)
    return y


def kernel(x, W_attn, b_attn, W_proj, b_proj, _trace=False):
    x = np.asarray(x, np.float32)
    W_attn = np.asarray(W_attn, np.float32)
    b_attn = np.asarray(b_attn, np.float32)
    W_proj = np.asarray(W_proj, np.float32)
    b_proj = np.asarray(b_proj, np.float32)

    nc = _get_module()
    in_maps = _make_in_maps(x, W_attn, b_attn, W_proj)
    kw = {}
    if _trace:
        _install_ntff_hook()
        kw = dict(trace=True)
    res = run_bass_kernel_spmd(nc, in_maps, core_ids=list(range(8)), **kw)
    out = _gather(res.results, b_proj)
    if _trace:
        return out, res
    return out


# revision 10
# speedup vs baseline: 1.5634x; 1.3422x over previous
"""Causal self-attention (B=2, T=2048, C=1024, H=16) on 8 Trainium2 NeuronCores.

Sharding (Megatron-style, per hint): core c handles batch b = c//4 and head
group g = c%4 (4 heads each).  c_attn is column-parallel (each core gets the
3x256 q/k/v columns for its heads), c_proj is row-parallel (each core gets
the 256 rows for its heads); the 4 partial outputs per batch are summed on
the host (the row-parallel all-reduce), plus b_proj.  Partials ship as bf16.

Single fused software-pipelined stream per core (keeps the PE HAM clock-gate
warm by never letting the Tensor engine idle during the ScalarE-bound
softmax phase):

  1. qkv projection for token block t4 is emitted as *filler* inside the
     attention j-loop of block t4-1, and the output projection of row block
     ib-1 likewise fills the attention of ib.  x^T is DMA'd in (co, t4)
     chunks so the first matmul starts ~2MB into the transfer.
  2. ST pair tiles [j, 2hp x i] live in one 2-bank PSUM tile so ONE ScalarE
     ACTIVATE (exp, fused 1/sqrt(D) scale) covers both heads of a pair --
     halves the fixed 352-cycle ACTIVATE overhead.  Causal masking is a
     multiplicative bf16 0/1 triangle on the post-exp tile (cheap DVE op).
  3. The two ST matmuls of a pair use K=64 row-halves of the PE array
     (distinct tile_position row groups) so they stream concurrently.
  4. YT[e, i] += V_aug^T @ P accumulates both heads into one 2-bank PSUM
     tile; row 64 is the softmax denominator (ones column of V_aug).
  5. Normalization: DVE reciprocal of the two denominator rows -> one bf16
     row, broadcast across 64 partitions by a K=1 PE matmul (two concurrent
     column-tiles), then a fused PSUMxPSUM multiply writes normalized Y^T
     straight to SBUF.  No DRAM round-trip.
"""

import os
import sys
import types
from collections import deque
from contextlib import ExitStack

import ml_dtypes
import numpy as np

for _p in ("/opt/trn_rl_repo",):
    if os.path.isdir(_p) and _p not in sys.path:
        sys.path.append(_p)
os.environ.setdefault("JAX_PLATFORMS", "cpu")

import concourse.bass as bass
import concourse.tile as tile
from concourse import bacc, mybir
from concourse.bass_utils import run_bass_kernel_spmd

B, T, C, H = 2, 2048, 1024, 16
P = 128
CO = C // P          # 8 contraction blocks for the qkv projection
HL = H // 4          # 4 local heads per core
D = C // H           # 64
F32 = mybir.dt.float32
BF16 = mybir.dt.bfloat16
EXPF = mybir.ActivationFunctionType.Exp
ADD = mybir.AluOpType.add
MULT = mybir.AluOpType.mult

_CACHE = {}


def _install_ntff_hook():
    """Agent image's antenv lacks axon_hooks; recreate so trace=True works."""
    try:
        from antenv import axon_hooks  # noqa: F401
        return
    except ImportError:
        pass
    try:
        import antenv
        from trn_agent_boot.trn_boot import _ntff_profile_via_ctypes
    except ImportError:
        return
    mod = types.ModuleType("antenv.axon_hooks")
    _hook = [None]
    mod.set_axon_ntff_profile_hook = lambda h: _hook.__setitem__(0, h)
    mod.get_axon_ntff_profile_hook = lambda: _hook[0]
    sys.modules["antenv.axon_hooks"] = mod
    antenv.axon_hooks = mod
    so = "/opt/axon/libaxon_pjrt.so"
    if os.path.exists(so):
        mod.set_axon_ntff_profile_hook(_ntff_profile_via_ctypes(so))


def build_module():
    nc = bacc.Bacc("TRN2", target_bir_lowering=False, debug=False, num_devices=8)

    # pre-swizzled on host so each DMA is one instruction with contiguous
    # per-partition lines: xt [p, t4, co, 512], w* [p, co, d], wp [p, ho, n]
    xt_d = nc.dram_tensor("xt", [P, T // 512, CO, 512], BF16,
                          kind="ExternalInput").ap()
    wq_d = nc.dram_tensor("wq", [P, CO, 256], BF16, kind="ExternalInput").ap()
    wk_d = nc.dram_tensor("wk", [P, CO, 256], BF16, kind="ExternalInput").ap()
    wv_d = nc.dram_tensor("wv", [P, CO, 256], BF16, kind="ExternalInput").ap()
    wp_d = nc.dram_tensor("wp", [P, 2, C], BF16, kind="ExternalInput").ap()
    bq_d = nc.dram_tensor("bq", [256], F32, kind="ExternalInput").ap()
    bk_d = nc.dram_tensor("bk", [256], F32, kind="ExternalInput").ap()
    bv_d = nc.dram_tensor("bv", [256], F32, kind="ExternalInput").ap()
    tri_d = nc.dram_tensor("tri", [P, 2 * P], BF16, kind="ExternalInput").ap()
    out_d = nc.dram_tensor("out", [T, C], BF16, kind="ExternalOutput").ap()

    NB = T // 512                       # 4 i-blocks of 512

    with tile.TileContext(nc) as tc, ExitStack() as ctx:
        const = ctx.enter_context(tc.tile_pool(name="const", bufs=1))
        s1w = ctx.enter_context(tc.tile_pool(name="s1w", bufs=1))
        # PSUM: 8 banks of [128, 512]f32.  psA(2) + psS(2x2) + psY(2) = 8.
        psA = ctx.enter_context(tc.tile_pool(name="psA", bufs=2, space="PSUM"))
        psS = ctx.enter_context(tc.tile_pool(name="psS", bufs=2, space="PSUM"))
        psY = ctx.enter_context(tc.tile_pool(name="psY", bufs=1, space="PSUM"))
        ppool = ctx.enter_context(tc.tile_pool(name="ppool", bufs=3))
        drp = ctx.enter_context(tc.tile_pool(name="drp", bufs=2))
        opool = ctx.enter_context(tc.tile_pool(name="opool", bufs=3))

        # ---- persistent SBUF tensors -------------------------------------
        qt = const.tile([P, 2, T], BF16, tag="qt")     # [d, ho, t]; head pair per ho
        kt = const.tile([P, 2, T], BF16, tag="kt")
        vsb = const.tile([P, T // P, HL, 66], BF16, tag="vsb")  # [tp, to, l, 1|V|1]
        yt2 = const.tile([P, 2, T], BF16, tag="yt2")   # normalized Y^T
        wp_sb = const.tile([P, 2, C], BF16, tag="wp")
        tri_sb = const.tile([P, 2 * P], BF16, tag="tri")   # 0/1 causal pair mask
        ones64 = const.tile([1, D], BF16, tag="ones64")
        bq_sb = const.tile([P, 2], F32, tag="bq")
        bk_sb = const.tile([P, 2], F32, tag="bk")
        bv_sb = const.tile([P, 256], F32, tag="bv")

        xt_sb = s1w.tile([P, CO, T], BF16, tag="xt")
        wq_sb = s1w.tile([P, CO, 256], BF16, tag="wq")
        wk_sb = s1w.tile([P, CO, 256], BF16, tag="wk")
        wv_sb = s1w.tile([P, CO, 256], BF16, tag="wv")

        # ---- input DMA, ordered by first use -----------------------------
        nc.sync.dma_start(tri_sb[:], tri_d)
        nc.sync.dma_start(bq_sb[:], bq_d.rearrange("(do p) -> p do", p=P))
        nc.sync.dma_start(bk_sb[:], bk_d.rearrange("(do p) -> p do", p=P))
        nc.sync.dma_start(wq_sb[:], wq_d)
        nc.sync.dma_start(wk_sb[:], wk_d)
        nc.sync.dma_start(xt_sb[:, :, 0:512], xt_d[:, 0])
        nc.sync.dma_start(wv_sb[:], wv_d)
        nc.sync.dma_start(
            bv_sb[:],
            bass.AP(tensor=bv_d.tensor, offset=bv_d.offset,
                    ap=[[0, P]] + list(bv_d.ap)),
        )
        for t4 in range(1, NB):
            nc.sync.dma_start(xt_sb[:, :, t4 * 512:(t4 + 1) * 512], xt_d[:, t4])
        nc.sync.dma_start(wp_sb[:], wp_d)
        nc.vector.memset(vsb[:, :, :, 65:66], 1.0)
        nc.vector.memset(ones64[:], 1.0)

        # ---- stage 1: qkv projection -------------------------------------
        def qk_group(w_sb, b_sb, dst, do, t4):
            # QT/KT d-major: psum[d, t] = W[:, dcols]^T @ x^T
            ps = psA.tile([P, 512], F32, tag="acc", name="qkps")
            for co in range(CO):
                nc.tensor.matmul(
                    ps[:],
                    lhsT=w_sb[:, co, do * P:(do + 1) * P],
                    rhs=xt_sb[:, co, t4 * 512:(t4 + 1) * 512],
                    start=(co == 0), stop=(co == CO - 1),
                )
            nc.vector.tensor_scalar_add(
                dst[:, do, t4 * 512:(t4 + 1) * 512], ps[:], b_sb[:, do:do + 1])

        def v_group(to):
            # V t-major: psum[t, d] = x^T-block^T @ Wv
            ps = psA.tile([P, 512], F32, tag="acc", name="vps")[:, 0:256]
            for co in range(CO):
                nc.tensor.matmul(
                    ps[:],
                    lhsT=xt_sb[:, co, to * P:(to + 1) * P],
                    rhs=wv_sb[:, co, :],
                    start=(co == 0), stop=(co == CO - 1),
                )
            nc.vector.tensor_tensor(
                vsb[:, to, :, 1:65],
                ps[:].rearrange("p (l e) -> p l e", l=HL),
                bv_sb[:].rearrange("p (l e) -> p l e", l=HL),
                op=ADD,
            )

        def qkv_emitters(t4):
            ems = []
            for do in range(2):
                ems.append(lambda do=do, t4=t4: qk_group(wq_sb, bq_sb, qt, do, t4))
                ems.append(lambda do=do, t4=t4: qk_group(wk_sb, bk_sb, kt, do, t4))
            for to in range(4 * t4, 4 * t4 + 4):
                ems.append(lambda to=to: v_group(to))
            return ems

        # ---- stage 5: output projection (row-parallel partial) -----------
        otiles = {}

        def proj_group(i1, nh):
            isl = slice(i1 * P, (i1 + 1) * P)
            nsl = slice(nh * 512, (nh + 1) * 512)
            ps = psA.tile([P, 512], F32, tag="acc", name="prps")
            for ho in range(2):
                nc.tensor.matmul(
                    ps[:], lhsT=yt2[:, ho, isl], rhs=wp_sb[:, ho, nsl],
                    start=(ho == 0), stop=(ho == 1))
            if nh == 0:
                otiles[i1] = opool.tile([P, C], BF16, tag="ot", name="ot")
            ot = otiles[i1]
            nc.vector.tensor_copy(ot[:, nsl], ps[:])
            if nh == 1:
                nc.sync.dma_start(out_d[isl, :], otiles.pop(i1)[:])

        def proj_emitters(ib):
            return [lambda i1=i1, nh=nh: proj_group(i1, nh)
                    for i1 in range(4 * ib, 4 * ib + 4) for nh in range(2)]

        # ---- stages 2-4: attention for head pair ho, row block ib --------
        tri3 = tri_sb.rearrange("p (h n) -> p h n", h=2)

        def attention(ho, ib, fillers, after_prologue=None):
            njb = 4 * ib + 4
            ytp = psY.tile([P, 1024], F32, tag="ytp")

            def win(jb):
                r = jb - 4 * ib
                i0 = jb * P if r >= 0 else ib * 512
                return r, i0, (ib + 1) * 512 - i0

            pts = {}

            def st_pair(jb):
                r, i0, N = win(jb)
                jsl = slice(jb * P, (jb + 1) * P)
                stp = psS.tile([P, 1024], F32, tag="stp")
                for hp in range(2):
                    pb = hp * 64
                    nc.tensor.matmul(
                        stp[:, hp * 512:hp * 512 + N],
                        lhsT=kt[pb:pb + 64, ho, jsl],
                        rhs=qt[pb:pb + 64, ho, i0:i0 + N],
                        start=True, stop=True)
                pt = ppool.tile([P, 2, 512], BF16, tag="pt")
                nc.scalar.activation(
                    pt[:, :, :N],
                    stp.rearrange("p (h n) -> p h n", h=2)[:, :, :N],
                    EXPF, scale=float(1.0 / np.sqrt(D)))
                if r >= 0:
                    nc.vector.tensor_tensor(
                        pt[:, :, 0:P], pt[:, :, 0:P], tri3[:], op=MULT)
                pts[jb] = pt

            def yt_pair(jb):
                _, i0, N = win(jb)
                f0 = i0 - ib * 512
                last = jb == njb - 1
                pt = pts.pop(jb)
                for hp in range(2):
                    nc.tensor.matmul(
                        ytp[0:65, hp * 512 + f0:hp * 512 + f0 + N],
                        lhsT=vsb[:, jb, 2 * ho + hp, 1:66],
                        rhs=pt[:, hp, :N], start=(jb == 0), stop=last)

            st_pair(0)
            if njb > 1:
                st_pair(1)
            if after_prologue is not None:
                after_prologue()
            for jb in range(njb):
                if jb + 2 < njb:
                    st_pair(jb + 2)
                if fillers:
                    fillers.popleft()()
                yt_pair(jb)

            def normalize():
                # denom row -> SBUF, K=1 matmul broadcasts it across 64
                # partitions per hp (concurrent col-tiles), then one WIDE
                # fast reciprocal over all 128 partitions (the narrow
                # nc.vector.reciprocal on [1, 1024] costs 6.5us and stalls
                # the in-order PE queue behind the broadcast).
                dsb = drp.tile([1, 1024], BF16, tag="dsb")
                nc.vector.tensor_copy(dsb[:], ytp[64:65, :])
                rps = psA.tile([P, 512], F32, tag="acc", name="rps")
                for hp in range(2):
                    nc.tensor.matmul(
                        rps[hp * 64:hp * 64 + 64, :],
                        lhsT=ones64[0:1, :],
                        rhs=dsb[0:1, hp * 512:(hp + 1) * 512],
                        start=True, stop=True)
                rsb = drp.tile([P, 512], F32, tag="rsb")
                nc.vector.reciprocal_approx_fast(rsb[:], rps[:])
                iw = slice(ib * 512, (ib + 1) * 512)
                for hp in range(2):
                    nc.vector.tensor_tensor(
                        yt2[hp * 64:hp * 64 + 64, ho, iw],
                        ytp[0:64, hp * 512:(hp + 1) * 512],
                        rsb[hp * 64:hp * 64 + 64, :], op=MULT)

            return normalize

        # ---- fused pipeline ----------------------------------------------
        for em in qkv_emitters(0):
            em()
        filler = deque()
        pending_norm = None
        for ib in range(NB):
            if ib + 1 < NB:
                filler.extend(qkv_emitters(ib + 1))
            if ib >= 1:
                filler.extend(proj_emitters(ib - 1))
            for ho in range(2):
                pending_norm = attention(ho, ib, filler,
                                         after_prologue=pending_norm)
            while filler:
                filler.popleft()()
        pending_norm()
        for em in proj_emitters(NB - 1):
            em()

    nc.compile()
    return nc


def _get_module():
    if "nc" not in _CACHE:
        _CACHE["nc"] = build_module()
    return _CACHE["nc"]


def _make_in_maps(x, W_attn, b_attn, W_proj):
    tri1 = np.where(np.arange(P)[None, :] >= np.arange(P)[:, None],
                    np.float32(1.0), np.float32(0.0))
    bf = ml_dtypes.bfloat16
    tri = np.concatenate([tri1, tri1], axis=1).astype(bf)

    def swz_w(w):        # [C, d] -> [p, co, d]
        return np.ascontiguousarray(w.reshape(CO, P, -1).transpose(1, 0, 2))

    def swz_xt(xb):      # [T, C] -> x^T as [p, t4, co, 512]
        return np.ascontiguousarray(
            xb.T.reshape(CO, P, T // 512, 512).transpose(1, 2, 0, 3))

    in_maps = []
    for core in range(8):
        b, g = divmod(core, 4)
        cs = slice(g * 256, (g + 1) * 256)
        in_maps.append({
            "xt": swz_xt(np.asarray(x[b])).astype(bf),
            "wq": swz_w(W_attn[:, g * 256:(g + 1) * 256]).astype(bf),
            "wk": swz_w(W_attn[:, C + g * 256:C + (g + 1) * 256]).astype(bf),
            "wv": swz_w(W_attn[:, 2 * C + g * 256:2 * C + (g + 1) * 256]).astype(bf),
            "wp": np.ascontiguousarray(
                W_proj[cs, :].reshape(2, P, C).transpose(1, 0, 2)).astype(bf),
            "bq": np.ascontiguousarray(b_attn[cs]),
            "bk": np.ascontiguousarray(b_attn[C + g * 256:C + (g + 1) * 256]),
            "bv": np.ascontiguousarray(b_attn[2 * C + g * 256:2 * C + (g + 1) * 256]),
            "tri": tri,
        })
    return in_maps


def _gather(results, b_proj):
    y = np.empty((B, T, C), np.float32)
    for b in range(B):
        acc = results[4 * b]["out"].astype(np.float32)
        for g in range(1, 4):
            acc = acc + results[4 * b + g]["out"].astype(np.float32)
        y[b] = acc + b_proj[None, :].astype(np.float32)
    return y


def kernel(x, W_attn, b_attn, W_proj, b_proj, _trace=False):
    x = np.asarray(x, np.float32)
    W_attn = np.asarray(W_attn, np.float32)
    b_attn = np.asarray(b_attn, np.float32)
    W_proj = np.asarray(W_proj, np.float32)
    b_proj = np.asarray(b_proj, np.float32)

    nc = _get_module()
    in_maps = _make_in_maps(x, W_attn, b_attn, W_proj)
    kw = {}
    if _trace:
        _install_ntff_hook()
        kw = dict(trace=True)
    res = run_bass_kernel_spmd(nc, in_maps, core_ids=list(range(8)), **kw)
    out = _gather(res.results, b_proj)
    if _trace:
        return out, res
    return out


# revision 18
# speedup vs baseline: 1.7156x; 1.0973x over previous
"""Causal self-attention (B=2, T=2048, C=1024, H=16) on 8 Trainium2 NeuronCores.

Sharding (Megatron-style, per hint): core c handles batch b = c//4 and head
group g = c%4 (4 heads each).  c_attn is column-parallel (each core gets the
3x256 q/k/v columns for its heads), c_proj is row-parallel (each core gets
the 256 rows for its heads); the 4 partial outputs per batch are summed on
the host (the row-parallel all-reduce), plus b_proj.  Partials ship as bf16.

Single fused software-pipelined stream per core (keeps the PE HAM clock-gate
warm by never letting the Tensor engine idle during the ScalarE-bound
softmax phase):

  1. qkv projection for token block t4 is emitted as *filler* inside the
     attention j-loop of block t4-1, and the output projection of row block
     ib-1 likewise fills the attention of ib.  x^T is DMA'd in (co, t4)
     chunks so the first matmul starts ~2MB into the transfer.
  2. ST pair tiles [j, 2hp x i] live in one 2-bank PSUM tile so ONE ScalarE
     ACTIVATE (exp, fused 1/sqrt(D) scale) covers both heads of a pair --
     halves the fixed 352-cycle ACTIVATE overhead.  Causal masking is a
     multiplicative bf16 0/1 triangle on the post-exp tile (cheap DVE op).
  3. The two ST matmuls of a pair use K=64 row-halves of the PE array
     (distinct tile_position row groups) so they stream concurrently.
  4. YT[e, i] += V_aug^T @ P accumulates both heads into one 2-bank PSUM
     tile; row 64 is the softmax denominator (ones column of V_aug).
  5. Normalization: DVE reciprocal of the two denominator rows -> one bf16
     row, broadcast across 64 partitions by a K=1 PE matmul (two concurrent
     column-tiles), then a fused PSUMxPSUM multiply writes normalized Y^T
     straight to SBUF.  No DRAM round-trip.
"""

import os
import sys
import types
from collections import deque
from contextlib import ExitStack

import ml_dtypes
import numpy as np

for _p in ("/opt/trn_rl_repo",):
    if os.path.isdir(_p) and _p not in sys.path:
        sys.path.append(_p)
os.environ.setdefault("JAX_PLATFORMS", "cpu")

import concourse.bass as bass
import concourse.tile as tile
from concourse import bacc, mybir
from concourse.bass_utils import run_bass_kernel_spmd

B, T, C, H = 2, 2048, 1024, 16
P = 128
CO = C // P          # 8 contraction blocks for the qkv projection
HL = H // 4          # 4 local heads per core
D = C // H           # 64
F32 = mybir.dt.float32
BF16 = mybir.dt.bfloat16
EXPF = mybir.ActivationFunctionType.Exp
ADD = mybir.AluOpType.add
MULT = mybir.AluOpType.mult

_CACHE = {}


def _install_ntff_hook():
    """Agent image's antenv lacks axon_hooks; recreate so trace=True works."""
    try:
        from antenv import axon_hooks  # noqa: F401
        return
    except ImportError:
        pass
    try:
        import antenv
        from trn_agent_boot.trn_boot import _ntff_profile_via_ctypes
    except ImportError:
        return
    mod = types.ModuleType("antenv.axon_hooks")
    _hook = [None]
    mod.set_axon_ntff_profile_hook = lambda h: _hook.__setitem__(0, h)
    mod.get_axon_ntff_profile_hook = lambda: _hook[0]
    sys.modules["antenv.axon_hooks"] = mod
    antenv.axon_hooks = mod
    so = "/opt/axon/libaxon_pjrt.so"
    if os.path.exists(so):
        mod.set_axon_ntff_profile_hook(_ntff_profile_via_ctypes(so))


def build_module():
    nc = bacc.Bacc("TRN2", target_bir_lowering=False, debug=False, num_devices=8)

    # pre-swizzled on host so each DMA is one instruction with contiguous
    # per-partition lines: xt [p, t4, co, 512], w* [p, co, d], wp [p, ho, n]
    xt_d = nc.dram_tensor("xt", [P, T // 512, CO, 512], BF16,
                          kind="ExternalInput").ap()
    wq_d = nc.dram_tensor("wq", [P, CO, 256], BF16, kind="ExternalInput").ap()
    wk_d = nc.dram_tensor("wk", [P, CO, 256], BF16, kind="ExternalInput").ap()
    wv_d = nc.dram_tensor("wv", [P, CO, 256], BF16, kind="ExternalInput").ap()
    wp_d = nc.dram_tensor("wp", [P, 2, C], BF16, kind="ExternalInput").ap()
    bq_d = nc.dram_tensor("bq", [256], F32, kind="ExternalInput").ap()
    bk_d = nc.dram_tensor("bk", [256], F32, kind="ExternalInput").ap()
    bv_d = nc.dram_tensor("bv", [256], F32, kind="ExternalInput").ap()
    tri_d = nc.dram_tensor("tri", [P, 2 * P], BF16, kind="ExternalInput").ap()
    out_d = nc.dram_tensor("out", [T, C], BF16, kind="ExternalOutput").ap()

    NB = T // 512                       # 4 i-blocks of 512

    with tile.TileContext(nc) as tc, ExitStack() as ctx:
        const = ctx.enter_context(tc.tile_pool(name="const", bufs=1))
        s1w = ctx.enter_context(tc.tile_pool(name="s1w", bufs=1))
        # PSUM: 8 banks of [128, 512]f32.  psA(2) + psS(2x2) + psY(2) = 8.
        psA = ctx.enter_context(tc.tile_pool(name="psA", bufs=2, space="PSUM"))
        psS = ctx.enter_context(tc.tile_pool(name="psS", bufs=2, space="PSUM"))
        psY = ctx.enter_context(tc.tile_pool(name="psY", bufs=1, space="PSUM"))
        ppool = ctx.enter_context(tc.tile_pool(name="ppool", bufs=3))
        drp = ctx.enter_context(tc.tile_pool(name="drp", bufs=2))
        opool = ctx.enter_context(tc.tile_pool(name="opool", bufs=3))

        # ---- persistent SBUF tensors -------------------------------------
        qt = const.tile([P, 2, T], BF16, tag="qt")     # [d, ho, t]; head pair per ho
        kt = const.tile([P, 2, T], BF16, tag="kt")
        vsb = const.tile([P, T // P, HL, 66], BF16, tag="vsb")  # [tp, to, l, 1|V|1]
        yt2 = const.tile([P, 2, T], BF16, tag="yt2")   # normalized Y^T
        wp_sb = const.tile([P, 2, C], BF16, tag="wp")
        tri_sb = const.tile([P, 2 * P], BF16, tag="tri")   # 0/1 causal pair mask
        ones64 = const.tile([1, D], BF16, tag="ones64")
        bq_sb = const.tile([P, 2], F32, tag="bq")
        bk_sb = const.tile([P, 2], F32, tag="bk")
        bv_sb = const.tile([P, 256], F32, tag="bv")

        xt_sb = s1w.tile([P, CO, T], BF16, tag="xt")
        wq_sb = s1w.tile([P, CO, 256], BF16, tag="wq")
        wk_sb = s1w.tile([P, CO, 256], BF16, tag="wk")
        wv_sb = s1w.tile([P, CO, 256], BF16, tag="wv")

        # ---- input DMA, ordered by first use -----------------------------
        nc.sync.dma_start(wq_sb[:], wq_d)
        nc.sync.dma_start(xt_sb[:, 0:4, 0:512], xt_d[:, 0, 0:4])
        nc.sync.dma_start(xt_sb[:, 4:8, 0:512], xt_d[:, 0, 4:8])
        nc.sync.dma_start(wk_sb[:], wk_d)
        nc.sync.dma_start(bq_sb[:], bq_d.rearrange("(do p) -> p do", p=P))
        nc.sync.dma_start(bk_sb[:], bk_d.rearrange("(do p) -> p do", p=P))
        nc.sync.dma_start(wv_sb[:], wv_d)
        nc.sync.dma_start(
            bv_sb[:],
            bass.AP(tensor=bv_d.tensor, offset=bv_d.offset,
                    ap=[[0, P]] + list(bv_d.ap)),
        )
        nc.sync.dma_start(tri_sb[:], tri_d)
        for t4 in range(1, NB):
            nc.sync.dma_start(xt_sb[:, :, t4 * 512:(t4 + 1) * 512], xt_d[:, t4])
        nc.sync.dma_start(wp_sb[:], wp_d)
        nc.vector.memset(vsb[:, :, :, 65:66], 1.0)
        nc.vector.memset(ones64[:], 1.0)

        # ---- stage 1: qkv projection -------------------------------------
        def qk_group(w_sb, b_sb, dst, do, t4):
            # QT/KT d-major: psum[d, t] = W[:, dcols]^T @ x^T
            ps = psA.tile([P, 512], F32, tag="acc", name="qkps")
            for co in range(CO):
                nc.tensor.matmul(
                    ps[:],
                    lhsT=w_sb[:, co, do * P:(do + 1) * P],
                    rhs=xt_sb[:, co, t4 * 512:(t4 + 1) * 512],
                    start=(co == 0), stop=(co == CO - 1),
                )
            nc.vector.tensor_scalar_add(
                dst[:, do, t4 * 512:(t4 + 1) * 512], ps[:], b_sb[:, do:do + 1])

        def v_group(to):
            # V t-major: psum[t, d] = x^T-block^T @ Wv
            ps = psA.tile([P, 512], F32, tag="acc", name="vps")[:, 0:256]
            for co in range(CO):
                nc.tensor.matmul(
                    ps[:],
                    lhsT=xt_sb[:, co, to * P:(to + 1) * P],
                    rhs=wv_sb[:, co, :],
                    start=(co == 0), stop=(co == CO - 1),
                )
            nc.vector.tensor_tensor(
                vsb[:, to, :, 1:65],
                ps[:].rearrange("p (l e) -> p l e", l=HL),
                bv_sb[:].rearrange("p (l e) -> p l e", l=HL),
                op=ADD,
            )

        def qk_emitters(t4):
            ems = []
            for do in range(2):
                ems.append(lambda do=do, t4=t4: qk_group(wq_sb, bq_sb, qt, do, t4))
                ems.append(lambda do=do, t4=t4: qk_group(wk_sb, bk_sb, kt, do, t4))
            return ems

        def v_emitters(t4):
            return [lambda to=to: v_group(to) for to in range(4 * t4, 4 * t4 + 4)]

        # ---- stage 5: output projection (row-parallel partial) -----------
        otiles = {}

        def proj_group(i1, nh):
            isl = slice(i1 * P, (i1 + 1) * P)
            nsl = slice(nh * 512, (nh + 1) * 512)
            ps = psA.tile([P, 512], F32, tag="acc", name="prps")
            for ho in range(2):
                nc.tensor.matmul(
                    ps[:], lhsT=yt2[:, ho, isl], rhs=wp_sb[:, ho, nsl],
                    start=(ho == 0), stop=(ho == 1))
            if nh == 0:
                otiles[i1] = opool.tile([P, C], BF16, tag="ot", name="ot")
            ot = otiles[i1]
            nc.vector.tensor_copy(ot[:, nsl], ps[:])
            if nh == 1:
                nc.sync.dma_start(out_d[isl, :], otiles.pop(i1)[:])

        def proj_emitters(ib):
            return [lambda i1=i1, nh=nh: proj_group(i1, nh)
                    for i1 in range(4 * ib, 4 * ib + 4) for nh in range(2)]

        # ---- stages 2-4: attention for head pair ho, row block ib --------
        tri3 = tri_sb.rearrange("p (h n) -> p h n", h=2)

        def attention(ho, ib, sched, after_prologue=None):
            njb = 4 * ib + 4
            ytp = psY.tile([P, 1024], F32, tag="ytp")

            def win(jb):
                r = jb - 4 * ib
                i0 = jb * P if r >= 0 else ib * 512
                return r, i0, (ib + 1) * 512 - i0

            pts = {}

            def st_pair(jb):
                r, i0, N = win(jb)
                jsl = slice(jb * P, (jb + 1) * P)
                stp = psS.tile([P, 1024], F32, tag="stp")
                for hp in range(2):
                    pb = hp * 64
                    nc.tensor.matmul(
                        stp[:, hp * 512:hp * 512 + N],
                        lhsT=kt[pb:pb + 64, ho, jsl],
                        rhs=qt[pb:pb + 64, ho, i0:i0 + N],
                        start=True, stop=True)
                pt = ppool.tile([P, 2, 512], BF16, tag="pt")
                nc.scalar.activation(
                    pt[:, :, :N],
                    stp.rearrange("p (h n) -> p h n", h=2)[:, :, :N],
                    EXPF, scale=float(1.0 / np.sqrt(D)))
                if r >= 0:
                    nc.vector.tensor_tensor(
                        pt[:, :, 0:P], pt[:, :, 0:P], tri3[:], op=MULT)
                pts[jb] = pt

            def yt_pair(jb):
                _, i0, N = win(jb)
                f0 = i0 - ib * 512
                last = jb == njb - 1
                pt = pts.pop(jb)
                for hp in range(2):
                    nc.tensor.matmul(
                        ytp[0:65, hp * 512 + f0:hp * 512 + f0 + N],
                        lhsT=vsb[:, jb, 2 * ho + hp, 1:66],
                        rhs=pt[:, hp, :N], start=(jb == 0), stop=last)

            st_pair(0)
            if njb > 1:
                st_pair(1)
            if after_prologue is not None:
                # previous block's denominator row -> SBUF (DVE), one filler
                # so the PE isn't stalled on it, then broadcast + normalize.
                # In a ho=0 prologue the filler may not be a proj group: the
                # pending normalize writes the yt2 rows proj reads.
                pre, post = after_prologue
                pre()
                sched.cover(allow_proj=(ho == 1))
                post()
            for jb in range(njb):
                if jb + 2 < njb:
                    st_pair(jb + 2)
                sched.step()
                yt_pair(jb)

            # normalize: denom row -> SBUF, K=1 matmul broadcasts it across
            # 64 partitions per hp (concurrent col-tiles), then one WIDE
            # fast reciprocal over all 128 partitions (the narrow
            # nc.vector.reciprocal on [1, 1024] costs 6.5us and stalls the
            # in-order PE queue behind the broadcast).
            dsb = drp.tile([1, 1024], BF16, tag="dsb")

            def norm_pre():
                nc.vector.tensor_copy(dsb[:], ytp[64:65, :])

            def norm_post():
                rps = psA.tile([P, 512], F32, tag="acc", name="rps")
                for hp in range(2):
                    nc.tensor.matmul(
                        rps[hp * 64:hp * 64 + 64, :],
                        lhsT=ones64[0:1, :],
                        rhs=dsb[0:1, hp * 512:(hp + 1) * 512],
                        start=True, stop=True)
                rsb = drp.tile([P, 512], F32, tag="rsb")
                nc.vector.reciprocal_approx_fast(rsb[:], rps[:])
                iw = slice(ib * 512, (ib + 1) * 512)
                for hp in range(2):
                    nc.vector.tensor_tensor(
                        yt2[hp * 64:hp * 64 + 64, ho, iw],
                        ytp[0:64, hp * 512:(hp + 1) * 512],
                        rsb[hp * 64:hp * 64 + 64, :], op=MULT)

            return norm_pre, norm_post

        # ---- fused pipeline ----------------------------------------------
        class _Sched:
            """PE filler scheduler for one row block's two attention calls.

            V-projection groups pop eagerly (their consumers are this same
            block's late YTs); qk/proj groups are paced evenly across the
            block's jb slots so the late, ScalarE-bound blocks keep the PE
            busy enough that the HAM clock-gate never re-throttles."""

            def __init__(self, v_ems, qk_ems, pr_ems, slots):
                self.v = deque(v_ems)
                self.qk = deque(qk_ems)
                self.pr = deque(pr_ems)
                self.slots = max(slots, 1)
                self.total = len(self.qk) + len(self.pr)
                self.done = 0
                self.slot = 0

            def _pop(self, allow_proj=True):
                if self.v:
                    self.v.popleft()()
                    return True
                if self.qk:
                    self.qk.popleft()()
                    self.done += 1
                    return True
                if allow_proj and self.pr:
                    self.pr.popleft()()
                    self.done += 1
                    return True
                return False

            def cover(self, allow_proj):
                self._pop(allow_proj)

            def step(self):
                self.slot += 1
                if self.v:
                    self.v.popleft()()
                    return
                target = -(-self.total * self.slot // self.slots)
                while self.done < target and (self.qk or self.pr):
                    self._pop(True)

            def flush(self):
                while self._pop(True):
                    pass

        for em in qk_emitters(0):
            em()
        for em in v_emitters(0):
            em()
        pending_norm = None
        for ib in range(NB):
            sched = _Sched(
                v_emitters(ib) if ib >= 1 else [],
                qk_emitters(ib + 1) if ib + 1 < NB else [],
                proj_emitters(ib - 1) if ib >= 1 else [],
                2 * (4 * ib + 4))
            for ho in range(2):
                pending_norm = attention(ho, ib, sched,
                                         after_prologue=pending_norm)
            sched.flush()
        pending_norm[0]()
        pending_norm[1]()
        for em in proj_emitters(NB - 1):
            em()

    nc.compile()
    return nc


def _get_module():
    if "nc" not in _CACHE:
        _CACHE["nc"] = build_module()
    return _CACHE["nc"]


def _make_in_maps(x, W_attn, b_attn, W_proj):
    tri1 = np.where(np.arange(P)[None, :] >= np.arange(P)[:, None],
                    np.float32(1.0), np.float32(0.0))
    bf = ml_dtypes.bfloat16
    tri = np.concatenate([tri1, tri1], axis=1).astype(bf)

    def swz_w(w):        # [C, d] -> [p, co, d]
        return np.ascontiguousarray(w.reshape(CO, P, -1).transpose(1, 0, 2))

    def swz_xt(xb):      # [T, C] -> x^T as [p, t4, co, 512]
        return np.ascontiguousarray(
            xb.T.reshape(CO, P, T // 512, 512).transpose(1, 2, 0, 3))

    in_maps = []
    for core in range(8):
        b, g = divmod(core, 4)
        cs = slice(g * 256, (g + 1) * 256)
        in_maps.append({
            "xt": swz_xt(np.asarray(x[b])).astype(bf),
            "wq": swz_w(W_attn[:, g * 256:(g + 1) * 256]).astype(bf),
            "wk": swz_w(W_attn[:, C + g * 256:C + (g + 1) * 256]).astype(bf),
            "wv": swz_w(W_attn[:, 2 * C + g * 256:2 * C + (g + 1) * 256]).astype(bf),
            "wp": np.ascontiguousarray(
                W_proj[cs, :].reshape(2, P, C).transpose(1, 0, 2)).astype(bf),
            "bq": np.ascontiguousarray(b_attn[cs]),
            "bk": np.ascontiguousarray(b_attn[C + g * 256:C + (g + 1) * 256]),
            "bv": np.ascontiguousarray(b_attn[2 * C + g * 256:2 * C + (g + 1) * 256]),
            "tri": tri,
        })
    return in_maps


def _gather(results, b_proj):
    y = np.empty((B, T, C), np.float32)
    for b in range(B):
        acc = results[4 * b]["out"].astype(np.float32)
        for g in range(1, 4):
            acc = acc + results[4 * b + g]["out"].astype(np.float32)
        y[b] = acc + b_proj[None, :].astype(np.float32)
    return y


def kernel(x, W_attn, b_attn, W_proj, b_proj, _trace=False):
    x = np.asarray(x, np.float32)
    W_attn = np.asarray(W_attn, np.float32)
    b_attn = np.asarray(b_attn, np.float32)
    W_proj = np.asarray(W_proj, np.float32)
    b_proj = np.asarray(b_proj, np.float32)

    nc = _get_module()
    in_maps = _make_in_maps(x, W_attn, b_attn, W_proj)
    kw = {}
    if _trace:
        _install_ntff_hook()
        kw = dict(trace=True)
    res = run_bass_kernel_spmd(nc, in_maps, core_ids=list(range(8)), **kw)
    out = _gather(res.results, b_proj)
    if _trace:
        return out, res
    return out
